# revision 7
# baseline (speedup 1.0000x reference)
"""RBF kernel layer (retrieval_knn): out = exp(-||x - p||^2) for x [131072, 64]
against 512 prototypes, distributed data-parallel over 8 NeuronCores.

v5 design (chunk-major, single fp16 GEMM, bf16 output):
  out[m, n] = exp(2 * (cross[m,n] - x_sq[n]/2 - p_sq[m]/2))
computed as ONE fp16 matmul per (128-proto tile, 512-point chunk) with K=68:
  lhsT = [p_t(64); 1; 1; -p_sq/2 hi; -p_sq/2 lo]   (stationary, 4 tiles)
  rhs  = [x_t(64); -x_sq/2 hi; -x_sq/2 lo; 1; 1]   (streamed, resident SBUF)
Loop is CHUNK-major (all 4 proto tiles per 512-point chunk) so each input
column is consumed 4x: the ACT stream needs only ~34 GB/s of input, which
the slow early DMA phase sustains -- ScalarE starts ~7us in and never
starves. exp is fused into the ACTIVATE (scale=2.0) over 4 PSUM banks at a
time (4+4 double buffer), emitted directly as bf16 (halves output DMA
bytes); host upconverts + transposes. x_sq/p_sq come from the QUANTIZED
fp16 inputs, so the kernel is the exact RBF of (x16, p16) and the error
(~2|x-p|*q) is smallest exactly where the output is largest.

Per-core budget (16384 points x 512 protos): ScalarE exp bound: 65536
elem/lane / 1.2GHz + 33 ACT overheads ~= 63us busy, back-to-back. DMA 2.2MB
in + 16.8MB out ~= 267 GB/s sustained (= ACT production rate). PE streams
128 matmuls of 512 cols (well under ScalarE). DVE idle.
"""

import numpy as np

# Problem constants (hardcoded per harness contract; kernel.py is self-contained)
N = 131072
D = 64
M = 512
GAMMA = 1.0
NCORES = 8
NSHARD = N // NCORES  # 16384
P = 128
K1 = D + 4  # 64 x rows + 2 (-x_sq/2 hi/lo) rows + 2 ones rows
MT = M // P  # 4 prototype tiles
NT = NSHARD // 512  # 32 column chunks of 512 points
OCHUNK = 4  # PSUM banks per ACTIVATE + output DMA (double-buffered 4+4)

_cache = {}


def _build_bass(nshard=NSHARD):
    import concourse.mybir as mybir
    import concourse.tile as tile
    from concourse import bacc

    f16 = mybir.dt.float16
    f32 = mybir.dt.float32
    bf16 = mybir.dt.bfloat16

    nc = bacc.Bacc(None, target_bir_lowering=False)
    # x pre-transposed on host: rows 0..63 features, 64/65 = -x_sq/2 hi/lo,
    # 66/67 = ones
    xr_d = nc.dram_tensor("xr", [K1, nshard], f16, kind="ExternalInput")
    # prototypes: rows 0..63 features, 64/65 = ones, 66/67 = -p_sq/2 hi/lo
    lhs_d = nc.dram_tensor("lhs", [K1, M], f16, kind="ExternalInput")
    # output transposed: out_t[m, n]; host converts to [n, m] f32
    out_d = nc.dram_tensor("out", [M, nshard], bf16, kind="ExternalOutput")

    with tile.TileContext(nc) as tc:
        with (
            tc.tile_pool(name="singles", bufs=1) as singles,
            tc.tile_pool(name="outp", bufs=4) as outp,
            tc.tile_pool(name="ps_o", bufs=2, space="PSUM") as ps_o,
        ):
            lhs_sb = singles.tile([K1, M], f16)
            nc.sync.dma_start(lhs_sb[:], lhs_d[:])

            # x resident in SBUF (2.2MB); ramped chunks: the first 512-col
            # chunk lands ~6us in so compute starts early, and the stream
            # only needs ~34 GB/s thereafter.
            X_sb = singles.tile([K1, nshard], f16)
            pos = 0
            for ch in (512, 512, 1024, 2048, 4096, 4096, 4096):
                nc.sync.dma_start(
                    X_sb[:, pos : pos + ch], xr_d[:, pos : pos + ch]
                )
                pos += ch
            assert pos == nshard

            for c in range(NT):
                rhs_ap = X_sb[:, c * 512 : (c + 1) * 512]
                psum = ps_o.tile([P, OCHUNK, 512], f32, tag="psum")
                o_sb = outp.tile([P, OCHUNK, 512], bf16, tag="o")
                for mt in range(MT):
                    nc.tensor.matmul(
                        psum[:, mt, :],
                        lhs_sb[:, mt * P : (mt + 1) * P],
                        rhs_ap,
                        start=True,
                        stop=True,
                    )
                # out = exp(2*S); the very last group is split in halves so
                # the final output DMA tail is shorter
                splits = [(0, 2), (2, 4)] if c == NT - 1 else [(0, OCHUNK)]
                for a, b in splits:
                    nc.scalar.activation(
                        o_sb[:, a:b, :],
                        psum[:, a:b, :],
                        mybir.ActivationFunctionType.Exp,
                        bias=0.0,
                        scale=2.0,
                    )
                    dest = out_d[
                        a * P : b * P, c * 512 : (c + 1) * 512
                    ].rearrange("(t p) m -> p t m", t=b - a)
                    nc.sync.dma_start(dest, o_sb[:, a:b, :])

    nc.finalize()
    return nc


def _get_nc():
    if "nc" not in _cache:
        _cache["nc"] = _build_bass()
    return _cache["nc"]


def _prep_core_arrays(x, prototypes, nshard):
    """Per-core host arrays: xr [68, nshard] f16, lhs [68, 512] f16."""
    x = np.ascontiguousarray(np.asarray(x, dtype=np.float32))
    prototypes = np.ascontiguousarray(np.asarray(prototypes, dtype=np.float32))

    x16 = x.astype(np.float16)
    p16 = prototypes.astype(np.float16)

    # -0.5 * squared norms of the QUANTIZED values, split hi/lo in fp16
    nxsq = (-0.5 * (x16.astype(np.float64) ** 2).sum(axis=1)).astype(np.float32)
    nxh = nxsq.astype(np.float16)
    nxl = (nxsq - nxh.astype(np.float32)).astype(np.float16)

    npsq = (-0.5 * (p16.astype(np.float64) ** 2).sum(axis=1)).astype(np.float32)
    nph = npsq.astype(np.float16)
    npl = (npsq - nph.astype(np.float32)).astype(np.float16)

    ones = np.ones((1, M), dtype=np.float16)
    lhs = np.ascontiguousarray(
        np.concatenate([p16.T, ones, ones, nph[None, :], npl[None, :]], axis=0)
    )  # [68, 512]

    ncores = x.shape[0] // nshard
    in_maps = []
    for s in range(ncores):
        sl = slice(s * nshard, (s + 1) * nshard)
        xr = np.empty((K1, nshard), dtype=np.float16)
        xr[:D] = x16[sl].T
        xr[D] = nxh[sl]
        xr[D + 1] = nxl[sl]
        xr[D + 2 :] = 1.0
        in_maps.append({"xr": xr, "lhs": lhs})
    return in_maps


def _prep_inputs(x, prototypes):
    return _prep_core_arrays(x, prototypes, NSHARD)


def _run(inputs, trace=False):
    from concourse.bass_utils import run_bass_kernel_spmd

    in_maps = _prep_inputs(inputs["x"], inputs["prototypes"])
    nc = _get_nc()
    res = run_bass_kernel_spmd(
        nc, in_maps, core_ids=list(range(NCORES)), trace=trace
    )
    out = np.empty((N, M), dtype=np.float32)
    for s, r in enumerate(res.results):
        # r["out"] is [512, 16384] bf16 -> [16384, 512] f32
        out[s * NSHARD : (s + 1) * NSHARD] = r["out"].astype(np.float32).T
    return out, res


def kernel(**inputs) -> np.ndarray:
    out, _ = _run(inputs, trace=False)
    return out


# revision 9
# speedup vs baseline: 1.0257x; 1.0257x over previous
"""RBF kernel layer (retrieval_knn): out = exp(-||x - p||^2) for x [131072, 64]
against 512 prototypes, distributed data-parallel over 8 NeuronCores.

v5 design (chunk-major, single fp16 GEMM, bf16 output):
  out[m, n] = exp(2 * (cross[m,n] - x_sq[n]/2 - p_sq[m]/2))
computed as ONE fp16 matmul per (128-proto tile, 512-point chunk) with K=68:
  lhsT = [p_t(64); 1; 1; -p_sq/2 hi; -p_sq/2 lo]   (stationary, 4 tiles)
  rhs  = [x_t(64); -x_sq/2 hi; -x_sq/2 lo; 1; 1]   (streamed, resident SBUF)
Loop is CHUNK-major (all 4 proto tiles per 512-point chunk) so each input
column is consumed 4x: the ACT stream needs only ~34 GB/s of input, which
the slow early DMA phase sustains -- ScalarE starts ~7us in and never
starves. exp is fused into the ACTIVATE (scale=2.0) over 4 PSUM banks at a
time (4+4 double buffer), emitted directly as bf16 (halves output DMA
bytes); host upconverts + transposes. x_sq/p_sq come from the QUANTIZED
fp16 inputs, so the kernel is the exact RBF of (x16, p16) and the error
(~2|x-p|*q) is smallest exactly where the output is largest.

Per-core budget (16384 points x 512 protos): ScalarE exp bound: 65536
elem/lane / 1.2GHz + 33 ACT overheads ~= 63us busy, back-to-back. DMA 2.2MB
in + 16.8MB out ~= 267 GB/s sustained (= ACT production rate). PE streams
128 matmuls of 512 cols (well under ScalarE). DVE idle.
"""

import numpy as np

# Problem constants (hardcoded per harness contract; kernel.py is self-contained)
N = 131072
D = 64
M = 512
GAMMA = 1.0
NCORES = 8
NSHARD = N // NCORES  # 16384
P = 128
K1 = D + 4  # 64 x rows + 2 (-x_sq/2 hi/lo) rows + 2 ones rows
MT = M // P  # 4 prototype tiles
NT = NSHARD // 512  # 32 column chunks of 512 points
OCHUNK = 4  # PSUM banks per ACTIVATE + output DMA (double-buffered 4+4)

_cache = {}


def _build_bass(nshard=NSHARD):
    import concourse.mybir as mybir
    import concourse.tile as tile
    from concourse import bacc

    f16 = mybir.dt.float16
    f32 = mybir.dt.float32
    bf16 = mybir.dt.bfloat16

    nc = bacc.Bacc(None, target_bir_lowering=False)
    # x pre-transposed on host: rows 0..63 features, 64/65 = -x_sq/2 hi/lo,
    # 66/67 = ones
    xr_d = nc.dram_tensor("xr", [K1, nshard], f16, kind="ExternalInput")
    # prototypes: rows 0..63 features, 64/65 = ones, 66/67 = -p_sq/2 hi/lo
    lhs_d = nc.dram_tensor("lhs", [K1, M], f16, kind="ExternalInput")
    # output transposed: out_t[m, n]; host converts to [n, m] f32
    out_d = nc.dram_tensor("out", [M, nshard], bf16, kind="ExternalOutput")

    with tile.TileContext(nc) as tc:
        with (
            tc.tile_pool(name="singles", bufs=1) as singles,
            tc.tile_pool(name="outp", bufs=6) as outp,
            tc.tile_pool(name="ps_o", bufs=2, space="PSUM") as ps_o,
        ):
            # Queue separation (the HWDGE queue is keyed to the issuing
            # engine): sync carries only the small leading x chunks and then
            # the output stream -- putting bulk input there head-of-line
            # blocks output and stalls the ACT pipeline. lhs goes on the
            # scalar queue (scalar idles until its first ACT anyway) and the
            # bulk x chunks on the gpsimd queue, so all three ramp their
            # serial descriptor-fetch phase in parallel.
            lhs_sb = singles.tile([K1, M], f16)
            nc.scalar.dma_start(lhs_sb[:], lhs_d[:])

            # x resident in SBUF (2.2MB); chunk-major consumption only needs
            # ~34 GB/s so the early trickle keeps the ACT stream fed.
            X_sb = singles.tile([K1, nshard], f16)
            pos = 0
            for ch, eng in (
                (512, nc.sync),
                (512, nc.sync),
                (1024, nc.sync),
                (2048, nc.gpsimd),
                (4096, nc.gpsimd),
                (4096, nc.gpsimd),
                (4096, nc.gpsimd),
            ):
                eng.dma_start(X_sb[:, pos : pos + ch], xr_d[:, pos : pos + ch])
                pos += ch
            assert pos == nshard

            for c in range(NT):
                rhs_ap = X_sb[:, c * 512 : (c + 1) * 512]
                psum = ps_o.tile([P, OCHUNK, 512], f32, tag="psum")
                o_sb = outp.tile([P, OCHUNK, 512], bf16, tag="o")
                for mt in range(MT):
                    nc.tensor.matmul(
                        psum[:, mt, :],
                        lhs_sb[:, mt * P : (mt + 1) * P],
                        rhs_ap,
                        start=True,
                        stop=True,
                    )
                # out = exp(2*S); the very last group is split in halves so
                # the final output DMA tail is shorter
                splits = [(0, 2), (2, 4)] if c == NT - 1 else [(0, OCHUNK)]
                for a, b in splits:
                    nc.scalar.activation(
                        o_sb[:, a:b, :],
                        psum[:, a:b, :],
                        mybir.ActivationFunctionType.Exp,
                        bias=0.0,
                        scale=2.0,
                    )
                    dest = out_d[
                        a * P : b * P, c * 512 : (c + 1) * 512
                    ].rearrange("(t p) m -> p t m", t=b - a)
                    nc.sync.dma_start(dest, o_sb[:, a:b, :])

    nc.finalize()
    return nc


def _get_nc():
    if "nc" not in _cache:
        _cache["nc"] = _build_bass()
    return _cache["nc"]


def _prep_core_arrays(x, prototypes, nshard):
    """Per-core host arrays: xr [68, nshard] f16, lhs [68, 512] f16."""
    x = np.ascontiguousarray(np.asarray(x, dtype=np.float32))
    prototypes = np.ascontiguousarray(np.asarray(prototypes, dtype=np.float32))

    x16 = x.astype(np.float16)
    p16 = prototypes.astype(np.float16)

    # -0.5 * squared norms of the QUANTIZED values, split hi/lo in fp16
    nxsq = (-0.5 * (x16.astype(np.float64) ** 2).sum(axis=1)).astype(np.float32)
    nxh = nxsq.astype(np.float16)
    nxl = (nxsq - nxh.astype(np.float32)).astype(np.float16)

    npsq = (-0.5 * (p16.astype(np.float64) ** 2).sum(axis=1)).astype(np.float32)
    nph = npsq.astype(np.float16)
    npl = (npsq - nph.astype(np.float32)).astype(np.float16)

    ones = np.ones((1, M), dtype=np.float16)
    lhs = np.ascontiguousarray(
        np.concatenate([p16.T, ones, ones, nph[None, :], npl[None, :]], axis=0)
    )  # [68, 512]

    ncores = x.shape[0] // nshard
    in_maps = []
    for s in range(ncores):
        sl = slice(s * nshard, (s + 1) * nshard)
        xr = np.empty((K1, nshard), dtype=np.float16)
        xr[:D] = x16[sl].T
        xr[D] = nxh[sl]
        xr[D + 1] = nxl[sl]
        xr[D + 2 :] = 1.0
        in_maps.append({"xr": xr, "lhs": lhs})
    return in_maps


def _prep_inputs(x, prototypes):
    return _prep_core_arrays(x, prototypes, NSHARD)


def _run(inputs, trace=False):
    from concourse.bass_utils import run_bass_kernel_spmd

    in_maps = _prep_inputs(inputs["x"], inputs["prototypes"])
    nc = _get_nc()
    res = run_bass_kernel_spmd(
        nc, in_maps, core_ids=list(range(NCORES)), trace=trace
    )
    out = np.empty((N, M), dtype=np.float32)
    for s, r in enumerate(res.results):
        # r["out"] is [512, 16384] bf16 -> [16384, 512] f32
        out[s * NSHARD : (s + 1) * NSHARD] = r["out"].astype(np.float32).T
    return out, res


def kernel(**inputs) -> np.ndarray:
    out, _ = _run(inputs, trace=False)
    return out


# revision 11
# speedup vs baseline: 1.0503x; 1.0240x over previous
"""RBF kernel layer (retrieval_knn): out = exp(-||x - p||^2) for x [131072, 64]
against 512 prototypes, distributed data-parallel over 8 NeuronCores.

v5 design (chunk-major, single fp16 GEMM, bf16 output):
  out[m, n] = exp(2 * (cross[m,n] - x_sq[n]/2 - p_sq[m]/2))
computed as ONE fp16 matmul per (128-proto tile, 512-point chunk) with K=68:
  lhsT = [p_t(64); 1; 1; -p_sq/2 hi; -p_sq/2 lo]   (stationary, 4 tiles)
  rhs  = [x_t(64); -x_sq/2 hi; -x_sq/2 lo; 1; 1]   (streamed, resident SBUF)
Loop is CHUNK-major (all 4 proto tiles per 512-point chunk) so each input
column is consumed 4x: the ACT stream needs only ~34 GB/s of input, which
the slow early DMA phase sustains -- ScalarE starts ~7us in and never
starves. exp is fused into the ACTIVATE (scale=2.0) over 4 PSUM banks at a
time (4+4 double buffer), emitted directly as bf16 (halves output DMA
bytes); host upconverts + transposes. x_sq/p_sq come from the QUANTIZED
fp16 inputs, so the kernel is the exact RBF of (x16, p16) and the error
(~2|x-p|*q) is smallest exactly where the output is largest.

Per-core budget (16384 points x 512 protos): ScalarE exp bound: 65536
elem/lane / 1.2GHz + 33 ACT overheads ~= 63us busy, back-to-back. DMA 2.2MB
in + 16.8MB out ~= 267 GB/s sustained (= ACT production rate). PE streams
128 matmuls of 512 cols (well under ScalarE). DVE idle.
"""

import numpy as np

# Problem constants (hardcoded per harness contract; kernel.py is self-contained)
N = 131072
D = 64
M = 512
GAMMA = 1.0
NCORES = 8
NSHARD = N // NCORES  # 16384
P = 128
K1 = D + 4  # 64 x rows + 2 (-x_sq/2 hi/lo) rows + 2 ones rows
MT = M // P  # 4 prototype tiles
NT = NSHARD // 512  # 32 column chunks of 512 points
OCHUNK = 4  # PSUM banks per ACTIVATE + output DMA (double-buffered 4+4)

_cache = {}


def _build_bass(nshard=NSHARD):
    import concourse.mybir as mybir
    import concourse.tile as tile
    from concourse import bacc

    f16 = mybir.dt.float16
    f32 = mybir.dt.float32
    bf16 = mybir.dt.bfloat16

    nc = bacc.Bacc(None, target_bir_lowering=False)
    # x pre-transposed on host: rows 0..63 features, 64/65 = -x_sq/2 hi/lo,
    # 66/67 = ones
    xr_d = nc.dram_tensor("xr", [K1, nshard], f16, kind="ExternalInput")
    # prototypes: rows 0..63 features, 64/65 = ones, 66/67 = -p_sq/2 hi/lo
    lhs_d = nc.dram_tensor("lhs", [K1, M], f16, kind="ExternalInput")
    # output transposed: out_t[m, n]; host converts to [n, m] f32
    out_d = nc.dram_tensor("out", [M, nshard], bf16, kind="ExternalOutput")

    with tile.TileContext(nc) as tc:
        with (
            tc.tile_pool(name="singles", bufs=1) as singles,
            tc.tile_pool(name="outp", bufs=8) as outp,
            tc.tile_pool(name="ps_o", bufs=2, space="PSUM") as ps_o,
        ):
            # Everything rides the single sync HWDGE queue (cross-queue
            # writers to one tile serialize badly in the dep tracker, and
            # extra queues pay their own slow descriptor-ramp). Ramped x
            # chunks start compute early; chunk-major consumption needs only
            # ~34 GB/s so the early trickle keeps up, and outp bufs=8
            # absorbs the window where output DMAs queue behind the
            # remaining input.
            lhs_sb = singles.tile([K1, M], f16)
            nc.sync.dma_start(lhs_sb[:], lhs_d[:])

            X_sb = singles.tile([K1, nshard], f16)
            pos = 0
            for ch in (512, 512, 1024, 2048, 4096, 4096, 4096):
                nc.sync.dma_start(
                    X_sb[:, pos : pos + ch], xr_d[:, pos : pos + ch]
                )
                pos += ch
            assert pos == nshard

            for c in range(NT):
                rhs_ap = X_sb[:, c * 512 : (c + 1) * 512]
                psum = ps_o.tile([P, OCHUNK, 512], f32, tag="psum")
                o_sb = outp.tile([P, OCHUNK, 512], bf16, tag="o")
                for mt in range(MT):
                    nc.tensor.matmul(
                        psum[:, mt, :],
                        lhs_sb[:, mt * P : (mt + 1) * P],
                        rhs_ap,
                        start=True,
                        stop=True,
                    )
                # out = exp(2*S); the very last group is split in halves so
                # the final output DMA tail is shorter
                splits = [(0, 2), (2, 4)] if c == NT - 1 else [(0, OCHUNK)]
                for a, b in splits:
                    nc.scalar.activation(
                        o_sb[:, a:b, :],
                        psum[:, a:b, :],
                        mybir.ActivationFunctionType.Exp,
                        bias=0.0,
                        scale=2.0,
                    )
                    dest = out_d[
                        a * P : b * P, c * 512 : (c + 1) * 512
                    ].rearrange("(t p) m -> p t m", t=b - a)
                    nc.sync.dma_start(dest, o_sb[:, a:b, :])

    nc.finalize()
    return nc


def _get_nc():
    if "nc" not in _cache:
        _cache["nc"] = _build_bass()
    return _cache["nc"]


def _prep_core_arrays(x, prototypes, nshard):
    """Per-core host arrays: xr [68, nshard] f16, lhs [68, 512] f16."""
    x = np.ascontiguousarray(np.asarray(x, dtype=np.float32))
    prototypes = np.ascontiguousarray(np.asarray(prototypes, dtype=np.float32))

    x16 = x.astype(np.float16)
    p16 = prototypes.astype(np.float16)

    # -0.5 * squared norms of the QUANTIZED values, split hi/lo in fp16
    nxsq = (-0.5 * (x16.astype(np.float64) ** 2).sum(axis=1)).astype(np.float32)
    nxh = nxsq.astype(np.float16)
    nxl = (nxsq - nxh.astype(np.float32)).astype(np.float16)

    npsq = (-0.5 * (p16.astype(np.float64) ** 2).sum(axis=1)).astype(np.float32)
    nph = npsq.astype(np.float16)
    npl = (npsq - nph.astype(np.float32)).astype(np.float16)

    ones = np.ones((1, M), dtype=np.float16)
    lhs = np.ascontiguousarray(
        np.concatenate([p16.T, ones, ones, nph[None, :], npl[None, :]], axis=0)
    )  # [68, 512]

    ncores = x.shape[0] // nshard
    in_maps = []
    for s in range(ncores):
        sl = slice(s * nshard, (s + 1) * nshard)
        xr = np.empty((K1, nshard), dtype=np.float16)
        xr[:D] = x16[sl].T
        xr[D] = nxh[sl]
        xr[D + 1] = nxl[sl]
        xr[D + 2 :] = 1.0
        in_maps.append({"xr": xr, "lhs": lhs})
    return in_maps


def _prep_inputs(x, prototypes):
    return _prep_core_arrays(x, prototypes, NSHARD)


def _run(inputs, trace=False):
    from concourse.bass_utils import run_bass_kernel_spmd

    in_maps = _prep_inputs(inputs["x"], inputs["prototypes"])
    nc = _get_nc()
    res = run_bass_kernel_spmd(
        nc, in_maps, core_ids=list(range(NCORES)), trace=trace
    )
    out = np.empty((N, M), dtype=np.float32)
    for s, r in enumerate(res.results):
        # r["out"] is [512, 16384] bf16 -> [16384, 512] f32
        out[s * NSHARD : (s + 1) * NSHARD] = r["out"].astype(np.float32).T
    return out, res


def kernel(**inputs) -> np.ndarray:
    out, _ = _run(inputs, trace=False)
    return out


# revision 13
# speedup vs baseline: 1.1369x; 1.0825x over previous
"""RBF kernel layer (retrieval_knn): out = exp(-||x - p||^2) for x [131072, 64]
against 512 prototypes, distributed data-parallel over 8 NeuronCores.

v5 design (chunk-major, single fp16 GEMM, bf16 output):
  out[m, n] = exp(2 * (cross[m,n] - x_sq[n]/2 - p_sq[m]/2))
computed as ONE fp16 matmul per (128-proto tile, 512-point chunk) with K=68:
  lhsT = [p_t(64); 1; 1; -p_sq/2 hi; -p_sq/2 lo]   (stationary, 4 tiles)
  rhs  = [x_t(64); -x_sq/2 hi; -x_sq/2 lo; 1; 1]   (streamed, resident SBUF)
Loop is CHUNK-major (all 4 proto tiles per 512-point chunk) so each input
column is consumed 4x: the ACT stream needs only ~34 GB/s of input, which
the slow early DMA phase sustains -- ScalarE starts ~7us in and never
starves. exp is fused into the ACTIVATE (scale=2.0) over 4 PSUM banks at a
time (4+4 double buffer), emitted directly as bf16 (halves output DMA
bytes); host upconverts + transposes. x_sq/p_sq come from the QUANTIZED
fp16 inputs, so the kernel is the exact RBF of (x16, p16) and the error
(~2|x-p|*q) is smallest exactly where the output is largest.

Per-core budget (16384 points x 512 protos): ScalarE exp bound: 65536
elem/lane / 1.2GHz + 33 ACT overheads ~= 63us busy, back-to-back. DMA 2.2MB
in + 16.8MB out ~= 267 GB/s sustained (= ACT production rate). PE streams
128 matmuls of 512 cols (well under ScalarE). DVE idle.
"""

import numpy as np

# Problem constants (hardcoded per harness contract; kernel.py is self-contained)
N = 131072
D = 64
M = 512
GAMMA = 1.0
NCORES = 8
NSHARD = N // NCORES  # 16384
P = 128
K1 = D + 4  # 64 x rows + 2 (-x_sq/2 hi/lo) rows + 2 ones rows
MT = M // P  # 4 prototype tiles
NT = NSHARD // 512  # 32 column chunks of 512 points
OCHUNK = 4  # PSUM banks per ACTIVATE + output DMA (double-buffered 4+4)

_cache = {}


def _build_bass(nshard=NSHARD):
    import concourse.mybir as mybir
    import concourse.tile as tile
    from concourse import bacc

    f16 = mybir.dt.float16
    f32 = mybir.dt.float32
    bf16 = mybir.dt.bfloat16

    nc = bacc.Bacc(None, target_bir_lowering=False)
    # x pre-transposed on host: rows 0..63 features, 64/65 = -x_sq/2 hi/lo,
    # 66/67 = ones
    xr_d = nc.dram_tensor("xr", [K1, nshard], f16, kind="ExternalInput")
    # prototypes: rows 0..63 features, 64/65 = ones, 66/67 = -p_sq/2 hi/lo
    lhs_d = nc.dram_tensor("lhs", [K1, M], f16, kind="ExternalInput")
    # output transposed: out_t[m, n]; host converts to [n, m] f32
    out_d = nc.dram_tensor("out", [M, nshard], bf16, kind="ExternalOutput")

    with tile.TileContext(nc) as tc:
        with (
            tc.tile_pool(name="singles", bufs=1) as singles,
            tc.tile_pool(name="outp", bufs=8) as outp,
            tc.tile_pool(name="ps_o", bufs=2, space="PSUM") as ps_o,
        ):
            # The sync engine carries ONLY input: each engine is a serial
            # instruction stream, so an output dma_start blocking on its ACT
            # would also hold back every input issue queued after it (and
            # the tile scheduler does interleave them). Outputs go on the
            # gpsimd engine, whose natural pacing is the ACT stream. Ramped
            # x chunks start compute early; chunk-major consumption needs
            # only ~34 GB/s so the early trickle keeps up.
            lhs_sb = singles.tile([K1, M], f16)
            nc.sync.dma_start(lhs_sb[:], lhs_d[:])

            X_sb = singles.tile([K1, nshard], f16)
            pos = 0
            for ch in (512, 512, 1024, 2048, 4096, 4096, 4096):
                nc.sync.dma_start(
                    X_sb[:, pos : pos + ch], xr_d[:, pos : pos + ch]
                )
                pos += ch
            assert pos == nshard

            for c in range(NT):
                rhs_ap = X_sb[:, c * 512 : (c + 1) * 512]
                psum = ps_o.tile([P, OCHUNK, 512], f32, tag="psum")
                o_sb = outp.tile([P, OCHUNK, 512], bf16, tag="o")
                for mt in range(MT):
                    nc.tensor.matmul(
                        psum[:, mt, :],
                        lhs_sb[:, mt * P : (mt + 1) * P],
                        rhs_ap,
                        start=True,
                        stop=True,
                    )
                # out = exp(2*S); the very last group is split in halves so
                # the final output DMA tail is shorter
                splits = [(0, 2), (2, 4)] if c == NT - 1 else [(0, OCHUNK)]
                for a, b in splits:
                    nc.scalar.activation(
                        o_sb[:, a:b, :],
                        psum[:, a:b, :],
                        mybir.ActivationFunctionType.Exp,
                        bias=0.0,
                        scale=2.0,
                    )
                    dest = out_d[
                        a * P : b * P, c * 512 : (c + 1) * 512
                    ].rearrange("(t p) m -> p t m", t=b - a)
                    nc.gpsimd.dma_start(dest, o_sb[:, a:b, :])

    nc.finalize()
    return nc


def _get_nc():
    if "nc" not in _cache:
        _cache["nc"] = _build_bass()
    return _cache["nc"]


def _prep_core_arrays(x, prototypes, nshard):
    """Per-core host arrays: xr [68, nshard] f16, lhs [68, 512] f16."""
    x = np.ascontiguousarray(np.asarray(x, dtype=np.float32))
    prototypes = np.ascontiguousarray(np.asarray(prototypes, dtype=np.float32))

    x16 = x.astype(np.float16)
    p16 = prototypes.astype(np.float16)

    # -0.5 * squared norms of the QUANTIZED values, split hi/lo in fp16
    nxsq = (-0.5 * (x16.astype(np.float64) ** 2).sum(axis=1)).astype(np.float32)
    nxh = nxsq.astype(np.float16)
    nxl = (nxsq - nxh.astype(np.float32)).astype(np.float16)

    npsq = (-0.5 * (p16.astype(np.float64) ** 2).sum(axis=1)).astype(np.float32)
    nph = npsq.astype(np.float16)
    npl = (npsq - nph.astype(np.float32)).astype(np.float16)

    ones = np.ones((1, M), dtype=np.float16)
    lhs = np.ascontiguousarray(
        np.concatenate([p16.T, ones, ones, nph[None, :], npl[None, :]], axis=0)
    )  # [68, 512]

    ncores = x.shape[0] // nshard
    in_maps = []
    for s in range(ncores):
        sl = slice(s * nshard, (s + 1) * nshard)
        xr = np.empty((K1, nshard), dtype=np.float16)
        xr[:D] = x16[sl].T
        xr[D] = nxh[sl]
        xr[D + 1] = nxl[sl]
        xr[D + 2 :] = 1.0
        in_maps.append({"xr": xr, "lhs": lhs})
    return in_maps


def _prep_inputs(x, prototypes):
    return _prep_core_arrays(x, prototypes, NSHARD)


def _run(inputs, trace=False):
    from concourse.bass_utils import run_bass_kernel_spmd

    in_maps = _prep_inputs(inputs["x"], inputs["prototypes"])
    nc = _get_nc()
    res = run_bass_kernel_spmd(
        nc, in_maps, core_ids=list(range(NCORES)), trace=trace
    )
    out = np.empty((N, M), dtype=np.float32)
    for s, r in enumerate(res.results):
        # r["out"] is [512, 16384] bf16 -> [16384, 512] f32
        out[s * NSHARD : (s + 1) * NSHARD] = r["out"].astype(np.float32).T
    return out, res


def kernel(**inputs) -> np.ndarray:
    out, _ = _run(inputs, trace=False)
    return out


# revision 15
# speedup vs baseline: 1.1662x; 1.0257x over previous
"""RBF kernel layer (retrieval_knn): out = exp(-||x - p||^2) for x [131072, 64]
against 512 prototypes, distributed data-parallel over 8 NeuronCores.

v5 design (chunk-major, single fp16 GEMM, bf16 output):
  out[m, n] = exp(2 * (cross[m,n] - x_sq[n]/2 - p_sq[m]/2))
computed as ONE fp16 matmul per (128-proto tile, 512-point chunk) with K=68:
  lhsT = [p_t(64); 1; 1; -p_sq/2 hi; -p_sq/2 lo]   (stationary, 4 tiles)
  rhs  = [x_t(64); -x_sq/2 hi; -x_sq/2 lo; 1; 1]   (streamed, resident SBUF)
Loop is CHUNK-major (all 4 proto tiles per 512-point chunk) so each input
column is consumed 4x: the ACT stream needs only ~34 GB/s of input, which
the slow early DMA phase sustains -- ScalarE starts ~7us in and never
starves. exp is fused into the ACTIVATE (scale=2.0) over 4 PSUM banks at a
time (4+4 double buffer), emitted directly as bf16 (halves output DMA
bytes); host upconverts + transposes. x_sq/p_sq come from the QUANTIZED
fp16 inputs, so the kernel is the exact RBF of (x16, p16) and the error
(~2|x-p|*q) is smallest exactly where the output is largest.

Per-core budget (16384 points x 512 protos): ScalarE exp bound: 65536
elem/lane / 1.2GHz + 33 ACT overheads ~= 63us busy, back-to-back. DMA 2.2MB
in + 16.8MB out ~= 267 GB/s sustained (= ACT production rate). PE streams
128 matmuls of 512 cols (well under ScalarE). DVE idle.
"""

import numpy as np

# Problem constants (hardcoded per harness contract; kernel.py is self-contained)
N = 131072
D = 64
M = 512
GAMMA = 1.0
NCORES = 8
NSHARD = N // NCORES  # 16384
P = 128
K1 = D + 4  # 64 x rows + 2 (-x_sq/2 hi/lo) rows + 2 ones rows
MT = M // P  # 4 prototype tiles
NT = NSHARD // 512  # 32 column chunks of 512 points
OCHUNK = 4  # PSUM banks per ACTIVATE + output DMA (double-buffered 4+4)

_cache = {}


def _build_bass(nshard=NSHARD):
    import concourse.mybir as mybir
    import concourse.tile as tile
    from concourse import bacc

    f16 = mybir.dt.float16
    f32 = mybir.dt.float32
    bf16 = mybir.dt.bfloat16

    nc = bacc.Bacc(None, target_bir_lowering=False)
    # x pre-transposed on host: rows 0..63 features, 64/65 = -x_sq/2 hi/lo,
    # 66/67 = ones
    xr_d = nc.dram_tensor("xr", [K1, nshard], f16, kind="ExternalInput")
    # prototypes: rows 0..63 features, 64/65 = ones, 66/67 = -p_sq/2 hi/lo
    lhs_d = nc.dram_tensor("lhs", [K1, M], f16, kind="ExternalInput")
    # output transposed: out_t[m, n]; host converts to [n, m] f32
    out_d = nc.dram_tensor("out", [M, nshard], bf16, kind="ExternalOutput")

    with tile.TileContext(nc) as tc:
        with (
            tc.tile_pool(name="singles", bufs=1) as singles,
            tc.tile_pool(name="outp", bufs=8) as outp,
            tc.tile_pool(name="ps_o", bufs=2, space="PSUM") as ps_o,
        ):
            # The sync engine carries ONLY input: each engine is a serial
            # instruction stream, so an output dma_start blocking on its ACT
            # would also hold back every input issue queued after it (and
            # the tile scheduler does interleave them). Outputs go on the
            # gpsimd engine, whose natural pacing is the ACT stream. Ramped
            # x chunks start compute early; chunk-major consumption needs
            # only ~34 GB/s so the early trickle keeps up.
            # Warm the gpsimd HWDGE queue before the first real output needs
            # it (a cold queue adds ~3.5us of ramp to the output stream).
            warm_sb = singles.tile([K1, 16], f16)
            nc.gpsimd.dma_start(warm_sb[:], xr_d[:, 0:16])

            lhs_sb = singles.tile([K1, M], f16)
            nc.sync.dma_start(lhs_sb[:], lhs_d[:])

            # 2048-col mid chunks: completion-event granularity is the whole
            # DMA, and consumption (~34 GB/s) outruns a late 4096-col chunk.
            X_sb = singles.tile([K1, nshard], f16)
            pos = 0
            for ch in (512, 512, 1024) + (2048,) * 7:
                nc.sync.dma_start(
                    X_sb[:, pos : pos + ch], xr_d[:, pos : pos + ch]
                )
                pos += ch
            assert pos == nshard

            for c in range(NT):
                rhs_ap = X_sb[:, c * 512 : (c + 1) * 512]
                psum = ps_o.tile([P, OCHUNK, 512], f32, tag="psum")
                o_sb = outp.tile([P, OCHUNK, 512], bf16, tag="o")
                for mt in range(MT):
                    nc.tensor.matmul(
                        psum[:, mt, :],
                        lhs_sb[:, mt * P : (mt + 1) * P],
                        rhs_ap,
                        start=True,
                        stop=True,
                    )
                # out = exp(2*S); the last two groups are split in halves so
                # the final output DMA tail is shorter
                splits = [(0, 2), (2, 4)] if c >= NT - 2 else [(0, OCHUNK)]
                for a, b in splits:
                    nc.scalar.activation(
                        o_sb[:, a:b, :],
                        psum[:, a:b, :],
                        mybir.ActivationFunctionType.Exp,
                        bias=0.0,
                        scale=2.0,
                    )
                    dest = out_d[
                        a * P : b * P, c * 512 : (c + 1) * 512
                    ].rearrange("(t p) m -> p t m", t=b - a)
                    nc.gpsimd.dma_start(dest, o_sb[:, a:b, :])

    nc.finalize()
    return nc


def _get_nc():
    if "nc" not in _cache:
        _cache["nc"] = _build_bass()
    return _cache["nc"]


def _prep_core_arrays(x, prototypes, nshard):
    """Per-core host arrays: xr [68, nshard] f16, lhs [68, 512] f16."""
    x = np.ascontiguousarray(np.asarray(x, dtype=np.float32))
    prototypes = np.ascontiguousarray(np.asarray(prototypes, dtype=np.float32))

    x16 = x.astype(np.float16)
    p16 = prototypes.astype(np.float16)

    # -0.5 * squared norms of the QUANTIZED values, split hi/lo in fp16
    nxsq = (-0.5 * (x16.astype(np.float64) ** 2).sum(axis=1)).astype(np.float32)
    nxh = nxsq.astype(np.float16)
    nxl = (nxsq - nxh.astype(np.float32)).astype(np.float16)

    npsq = (-0.5 * (p16.astype(np.float64) ** 2).sum(axis=1)).astype(np.float32)
    nph = npsq.astype(np.float16)
    npl = (npsq - nph.astype(np.float32)).astype(np.float16)

    ones = np.ones((1, M), dtype=np.float16)
    lhs = np.ascontiguousarray(
        np.concatenate([p16.T, ones, ones, nph[None, :], npl[None, :]], axis=0)
    )  # [68, 512]

    ncores = x.shape[0] // nshard
    in_maps = []
    for s in range(ncores):
        sl = slice(s * nshard, (s + 1) * nshard)
        xr = np.empty((K1, nshard), dtype=np.float16)
        xr[:D] = x16[sl].T
        xr[D] = nxh[sl]
        xr[D + 1] = nxl[sl]
        xr[D + 2 :] = 1.0
        in_maps.append({"xr": xr, "lhs": lhs})
    return in_maps


def _prep_inputs(x, prototypes):
    return _prep_core_arrays(x, prototypes, NSHARD)


def _run(inputs, trace=False):
    from concourse.bass_utils import run_bass_kernel_spmd

    in_maps = _prep_inputs(inputs["x"], inputs["prototypes"])
    nc = _get_nc()
    res = run_bass_kernel_spmd(
        nc, in_maps, core_ids=list(range(NCORES)), trace=trace
    )
    out = np.empty((N, M), dtype=np.float32)
    for s, r in enumerate(res.results):
        # r["out"] is [512, 16384] bf16 -> [16384, 512] f32
        out[s * NSHARD : (s + 1) * NSHARD] = r["out"].astype(np.float32).T
    return out, res


def kernel(**inputs) -> np.ndarray:
    out, _ = _run(inputs, trace=False)
    return out
